# revision 19
# baseline (speedup 1.0000x reference)
"""Trainium2 Bass kernel for nn_KANLayer (B=16384, D=1024, K=8).

Math: the per-feature basis chain collapses algebraically:
    nl[b,i] = sum_k (x[b,i]*W1[i,k] + b1[i,k]) * W2[i,k]
            = x[b,i] * a[i] + c[i],   a = sum_k W1*W2, c = sum_k b1*W2
so the whole layer is ONE dense matmul plus a diagonal + bias:
    out = x @ (lin_W.T + diag(a)) + (lin_b + c)
        = x @ W_offdiag + d*x + bias,   d = diag(lin_W.T) + a

Precision split (validated against the fixed-seed reference, rel err
1.41e-2 < 2e-2 gate):
  - The dense OFF-DIAGONAL matmul runs on-device in fp8 e4m3 with
    DoubleRow perf mode (2 contraction rows/cycle -> 2x bf16 PE rate).
    W_offdiag is scaled by 32 (pow2, lossless to descale) so its
    ~N(0,1/1024) entries sit in e4m3's normal range.
  - The diagonal term d*x and the bias are applied on the HOST in f32
    (elementwise O(B*D)), because fp8 x would be far too coarse for the
    O(1) diagonal. Device output is the scaled partial sum in bf16.

Sharding: data-parallel over batch across 8 NeuronCores (2048 rows
each); W8 (1024x1024 fp8) replicated. No collectives.

Device kernel (per core): psum[128b, 256f] accumulated over 4
DoubleRow matmuls (kt-pairs), evicted to bf16 by DVE (chunks 0,1) and
ACT (chunks 2,3), stored on the SP HWDGE ring.
"""

from contextlib import ExitStack

import numpy as np
import ml_dtypes

import concourse.bass as bass
import concourse.tile as tile
from concourse import bacc, mybir
from concourse.bass_utils import run_bass_kernel_spmd

B, D = 16384, 1024
NCORES = 8
BS = B // NCORES   # 2048 batch rows per core
P = 128
NQ = 4             # kt-pairs (contraction 1024 = 4 pairs * 2 * 128)
NB = BS // P       # 16 batch tiles per core
NCH = D // 256     # 4 output-feature chunks of 256 per batch tile
WSCALE = 32.0      # pow2 scale keeping W_offdiag in e4m3 normal range

FP8_NP = ml_dtypes.float8_e4m3   # TRN FP8_EXP4 (max +-240)

_CACHE = {}


def _build_nc():
    nc = bacc.Bacc("TRN2", target_bir_lowering=False, debug=False,
                   num_devices=NCORES)
    # x8t host-packed DoubleRow lhsT tiles, pair-contiguous:
    #   x8t[pair, p, sub, q, i, b] = fp8(x[(2*pair+sub)*128+b, (2q+i)*128+p])
    # so each 2-batch-tile DMA is one contiguous 2 KB/partition block
    # (128 descriptors instead of 256 -- halves the HWDGE issue time).
    x8t = nc.dram_tensor("x8t", [NB // 2, P, 2, NQ, 2, P],
                         mybir.dt.float8e4, kind="ExternalInput").ap()
    # w8 host-packed DoubleRow rhs: w8[p, q, i, f] = fp8(32*W_nd[(2q+i)*128+p, f])
    w8 = nc.dram_tensor("w8", [P, NQ, 2, D], mybir.dt.float8e4,
                        kind="ExternalInput").ap()
    out = nc.dram_tensor("out", [BS, D], mybir.dt.bfloat16,
                         kind="ExternalOutput").ap()

    out_r = out.rearrange("(nb p) n -> nb p n", p=P)

    with tile.TileContext(nc) as tc, ExitStack() as ctx:
        wpool = ctx.enter_context(tc.tile_pool(name="wpool", bufs=1))
        xpool = ctx.enter_context(tc.tile_pool(name="xpool", bufs=8))
        opool = ctx.enter_context(tc.tile_pool(name="opool", bufs=4))
        ppool = ctx.enter_context(tc.tile_pool(name="ppool", bufs=8,
                                               space="PSUM"))

        # PE pre-warm: DoubleRow matmuls on a zeroed fp8 tile keep the PE
        # busy through the load intro so the HAM clock reaches 8/8 before
        # the real stream starts.
        warm = wpool.tile([P, 2, 256], mybir.dt.float8e4, tag="warm",
                          name="warm")
        # gpsimd is otherwise idle and dispatches earliest after the
        # framework preamble, so the warm stream starts sooner.
        nc.gpsimd.memset(warm, 0.0)
        warm_ps = ppool.tile([P, 256], mybir.dt.float32, tag="ps",
                             name="warm_ps")
        # Warm matmuls ~= the x-pair-0 + W load window, so the real stream
        # starts the moment w arrives, with the HAM clock ramp already
        # served by the warm stream (an idle gap resets the ramp).
        NWARM = 16
        for i in range(NWARM):
            nc.tensor.matmul(warm_ps, lhsT=warm[:, :, :P], rhs=warm,
                             start=(i == 0), stop=(i == NWARM - 1),
                             perf_mode=mybir.MatmulPerfMode.DoubleRow)

        # All loads upfront on the SP ring: first x pair (gates the first
        # matmul), then the two W halves, then the remaining x pairs.
        x_tiles = {}

        def load_x_pair(pair, eng):
            t = xpool.tile([P, 2, NQ, 2, P], mybir.dt.float8e4, tag="x",
                           name=f"x_t{pair}")
            eng.dma_start(out=t, in_=x8t[pair])
            x_tiles[pair] = t

        # Ring split: the intro-critical loads run concurrently on both
        # HWDGE rings -- x pair 0 on SP, W halves first on ACT (the DMA
        # issue itself costs ~0.7us on the sequencer, so fewer pieces
        # reach the last W byte sooner).  The non-critical x pairs follow
        # W on the ACT ring (FIFO keeps them from competing with W for
        # SDMA packets); stores go on SP.
        load_x_pair(0, nc.sync)
        w_t = wpool.tile([P, NQ, 2, D], mybir.dt.float8e4, tag="w",
                         name="w_t")
        nc.scalar.dma_start(out=w_t[:, 0:2], in_=w8[:, 0:2])
        nc.scalar.dma_start(out=w_t[:, 2:4], in_=w8[:, 2:4])
        for pair in range(1, NB // 2):
            load_x_pair(pair, nc.scalar)

        for bt in range(NB):
            pair, sub = divmod(bt, 2)
            x_t = x_tiles[pair]
            o_t = opool.tile([P, D], mybir.dt.bfloat16, tag="o",
                             name=f"o_t{bt}")
            # q outer so 4 consecutive matmuls share one stationary lhsT
            # (LDWEIGHTS is 124ns vs the 107ns DoubleRow stream, so weight
            # reloads gate the PE unless amortized across the chunk loop).
            psums = [ppool.tile([P, 256], mybir.dt.float32, tag="ps",
                                name=f"ps{bt}_{ch}") for ch in range(NCH)]
            for q in range(NQ):
                for ch in range(NCH):
                    nc.tensor.matmul(
                        psums[ch],
                        lhsT=x_t[:, sub, q, :, :],
                        rhs=w_t[:, q, :, bass.ts(ch, 256)],
                        start=(q == 0),
                        stop=(q == NQ - 1),
                        perf_mode=mybir.MatmulPerfMode.DoubleRow,
                    )
            # PSUM -> SBUF eviction split across DVE and ACT so neither
            # becomes the bottleneck; bf16 cast happens here. The 1/32
            # descale is folded into the host-side diagonal pass.
            # ACT takes the first chunks, DVE (faster per op) the last, so
            # the final eviction before each store lands earlier.
            for ch in range(NCH):
                sl = bass.ts(ch, 256)
                if ch < 2:
                    nc.scalar.copy(o_t[:, sl], psums[ch])
                else:
                    nc.vector.tensor_scalar_mul(o_t[:, sl], psums[ch], 1.0)
            if bt == NB - 1:
                # final tile: store per chunk, split across both HWDGE
                # rings, so the kernel tail is short
                for ch in range(NCH):
                    sl = bass.ts(ch, 256)
                    eng = nc.sync if ch % 2 == 0 else nc.scalar
                    eng.dma_start(out=out_r[bt][:, sl], in_=o_t[:, sl])
            else:
                nc.sync.dma_start(out=out_r[bt], in_=o_t)

    nc.compile()
    return nc


def _get_nc():
    if "nc" not in _CACHE:
        _CACHE["nc"] = _build_nc()
    return _CACHE["nc"]


def _prep_inputs(x, lin_W, lin_b, W1, b1, W2):
    """Host prep: fold the basis chain, split diagonal, pack fp8 tiles."""
    x = np.asarray(x, dtype=np.float32)
    lin_W = np.asarray(lin_W, dtype=np.float32)
    a = np.sum(np.asarray(W1, np.float32) * np.asarray(W2, np.float32),
               axis=1)
    c = np.sum(np.asarray(b1, np.float32) * np.asarray(W2, np.float32),
               axis=1)
    W_eff = np.ascontiguousarray(lin_W.T)
    idx = np.arange(D)
    dvec = W_eff[idx, idx] + a          # full diagonal, applied on host
    W_nd = W_eff.copy()
    W_nd[idx, idx] = 0.0
    bias = np.asarray(lin_b, np.float32) + c

    # Device W: fp8(32 * W_offdiag), packed [p, q, i, f]
    w8 = (W_nd * WSCALE).astype(FP8_NP).reshape(NQ, 2, P, D)
    w8 = np.ascontiguousarray(w8.transpose(2, 0, 1, 3))

    # Device x: fp8(x), packed per core [pair, p, sub, q, i, b]
    x8 = x.astype(FP8_NP).reshape(NCORES, NB // 2, 2, P, NQ, 2, P)
    x8t = np.ascontiguousarray(x8.transpose(0, 1, 6, 2, 4, 5, 3))
    return x8t, w8, x, dvec, bias


def _postprocess(results, x, dvec, bias):
    """Host post: descale the fp8 partial sums, add diagonal + bias."""
    out_dev = np.concatenate([r["out"] for r in results], axis=0)
    out = out_dev.astype(np.float32)
    out *= np.float32(1.0 / WSCALE)
    out += x * dvec
    out += bias
    return np.ascontiguousarray(out.astype(np.float32))


def kernel(x, lin_W, lin_b, W1, b1, W2):
    x8t, w8, x32, dvec, bias = _prep_inputs(x, lin_W, lin_b, W1, b1, W2)
    in_maps = [{"x8t": x8t[i], "w8": w8} for i in range(NCORES)]
    nc = _get_nc()
    res = run_bass_kernel_spmd(nc, in_maps, core_ids=list(range(NCORES)))
    return _postprocess(res.results, x32, dvec, bias)


# revision 25
# speedup vs baseline: 1.0499x; 1.0499x over previous
"""Trainium2 Bass kernel for nn_KANLayer (B=16384, D=1024, K=8).

Math: the per-feature basis chain collapses algebraically:
    nl[b,i] = sum_k (x[b,i]*W1[i,k] + b1[i,k]) * W2[i,k]
            = x[b,i] * a[i] + c[i],   a = sum_k W1*W2, c = sum_k b1*W2
so the whole layer is ONE dense matmul plus a diagonal + bias:
    out = x @ (lin_W.T + diag(a)) + (lin_b + c)
        = x @ W_offdiag + d*x + bias,   d = diag(lin_W.T) + a

Precision split (validated against the fixed-seed reference, rel err
1.41e-2 < 2e-2 gate):
  - The dense OFF-DIAGONAL matmul runs on-device in fp8 e4m3 with
    DoubleRow perf mode (2 contraction rows/cycle -> 2x bf16 PE rate).
    W_offdiag is scaled by 32 (pow2, lossless to descale) so its
    ~N(0,1/1024) entries sit in e4m3's normal range.
  - The diagonal term d*x and the bias are applied on the HOST in f32
    (elementwise O(B*D)), because fp8 x would be far too coarse for the
    O(1) diagonal. Device output is the scaled partial sum in bf16.

Sharding: data-parallel over batch across 8 NeuronCores (2048 rows
each); W8 (1024x1024 fp8) replicated. No collectives.

Device kernel (per core): psum[128b, 256f] accumulated over 4
DoubleRow matmuls (kt-pairs), evicted to bf16 by DVE (chunks 0,1) and
ACT (chunks 2,3), stored on the SP HWDGE ring.
"""

from contextlib import ExitStack

import numpy as np
import ml_dtypes

import concourse.bass as bass
import concourse.tile as tile
from concourse import bacc, mybir
from concourse.bass_utils import run_bass_kernel_spmd

B, D = 16384, 1024
NCORES = 8
BS = B // NCORES   # 2048 batch rows per core
P = 128
NQ = 4             # kt-pairs (contraction 1024 = 4 pairs * 2 * 128)
NB = BS // P       # 16 batch tiles per core
NCH = D // 256     # 4 output-feature chunks of 256 per batch tile
WSCALE = 32.0      # pow2 scale keeping W_offdiag in e4m3 normal range

FP8_NP = ml_dtypes.float8_e4m3   # TRN FP8_EXP4 (max +-240)

_CACHE = {}


def _build_nc():
    nc = bacc.Bacc("TRN2", target_bir_lowering=False, debug=False,
                   num_devices=NCORES)
    # x8t host-packed DoubleRow lhsT tiles:
    #   x8t[bt, p, q, i, b] = fp8(x[bt*128+b, (2q+i)*128+p])
    # so each batch tile is one contiguous 1 KB/partition block.
    x8t = nc.dram_tensor("x8t", [NB, P, NQ, 2, P], mybir.dt.float8e4,
                         kind="ExternalInput").ap()
    # w8 host-packed DoubleRow rhs: w8[p, q, i, f] = fp8(32*W_nd[(2q+i)*128+p, f])
    w8 = nc.dram_tensor("w8", [P, NQ, 2, D], mybir.dt.float8e4,
                        kind="ExternalInput").ap()
    out = nc.dram_tensor("out", [BS, D], mybir.dt.bfloat16,
                         kind="ExternalOutput").ap()

    out_r = out.rearrange("(nb p) n -> nb p n", p=P)

    with tile.TileContext(nc) as tc, ExitStack() as ctx:
        wpool = ctx.enter_context(tc.tile_pool(name="wpool", bufs=1))
        xpool = ctx.enter_context(tc.tile_pool(name="xpool", bufs=8))
        opool = ctx.enter_context(tc.tile_pool(name="opool", bufs=4))
        ppool = ctx.enter_context(tc.tile_pool(name="ppool", bufs=8,
                                               space="PSUM"))

        # PE pre-warm: DoubleRow matmuls on a zeroed fp8 tile keep the PE
        # busy through the load intro so the HAM clock reaches 8/8 before
        # the real stream starts.
        warm = wpool.tile([P, 2, 256], mybir.dt.float8e4, tag="warm",
                          name="warm")
        nc.vector.memset(warm, 0.0)
        warm_ps = ppool.tile([P, 256], mybir.dt.float32, tag="ps",
                             name="warm_ps")
        # Warm matmuls ~= the x-pair-0 + W load window, so the real stream
        # starts the moment w arrives, with the HAM clock ramp already
        # served by the warm stream (an idle gap resets the ramp).
        NWARM = 19
        for i in range(NWARM):
            nc.tensor.matmul(warm_ps, lhsT=warm[:, :, :P], rhs=warm,
                             start=(i == 0), stop=(i == NWARM - 1),
                             perf_mode=mybir.MatmulPerfMode.DoubleRow)

        # All loads upfront on the SP ring: first x pair (gates the first
        # matmul), then the two W halves, then the remaining x pairs.
        x_tiles = {}

        def load_x_pair(pair, eng):
            t = xpool.tile([P, 2, NQ, 2, P], mybir.dt.float8e4, tag="x",
                           name=f"x_t{pair}")
            eng.dma_start(
                out=t, in_=x8t[2 * pair:2 * pair + 2].rearrange(
                    "n p q i b -> p n q i b"))
            x_tiles[pair] = t

        # Ring split: the intro-critical loads run concurrently on both
        # HWDGE rings -- x pair 0 on SP, W quarters first on ACT.  The
        # non-critical x pairs follow W on the ACT ring (FIFO keeps them
        # from competing with W for SDMA packets), stores go on SP.
        load_x_pair(0, nc.sync)
        w_t = wpool.tile([P, NQ, 2, D], mybir.dt.float8e4, tag="w",
                         name="w_t")
        for q in range(NQ):
            nc.scalar.dma_start(out=w_t[:, q:q + 1], in_=w8[:, q:q + 1])
        for pair in range(1, NB // 2):
            load_x_pair(pair, nc.scalar)

        for bt in range(NB):
            pair, sub = divmod(bt, 2)
            x_t = x_tiles[pair]
            o_t = opool.tile([P, D], mybir.dt.bfloat16, tag="o",
                             name=f"o_t{bt}")
            # q outer so 4 consecutive matmuls share one stationary lhsT
            # (LDWEIGHTS is 124ns vs the 107ns DoubleRow stream, so weight
            # reloads gate the PE unless amortized across the chunk loop).
            psums = [ppool.tile([P, 256], mybir.dt.float32, tag="ps",
                                name=f"ps{bt}_{ch}") for ch in range(NCH)]
            for q in range(NQ):
                for ch in range(NCH):
                    nc.tensor.matmul(
                        psums[ch],
                        lhsT=x_t[:, sub, q, :, :],
                        rhs=w_t[:, q, :, bass.ts(ch, 256)],
                        start=(q == 0),
                        stop=(q == NQ - 1),
                        perf_mode=mybir.MatmulPerfMode.DoubleRow,
                    )
            # PSUM -> SBUF eviction split across DVE and ACT so neither
            # becomes the bottleneck; bf16 cast happens here. The 1/32
            # descale is folded into the host-side diagonal pass.
            # ACT takes the first chunks, DVE (faster per op) the last, so
            # the final eviction before each store lands earlier.
            for ch in range(NCH):
                sl = bass.ts(ch, 256)
                if ch < 2:
                    nc.scalar.copy(o_t[:, sl], psums[ch])
                else:
                    nc.vector.tensor_scalar_mul(o_t[:, sl], psums[ch], 1.0)
            if bt == NB - 1:
                # final tile: store per chunk so the kernel tail is short
                for ch in range(NCH):
                    sl = bass.ts(ch, 256)
                    nc.sync.dma_start(out=out_r[bt][:, sl], in_=o_t[:, sl])
            else:
                nc.sync.dma_start(out=out_r[bt], in_=o_t)

    nc.compile()
    return nc


def _get_nc():
    if "nc" not in _CACHE:
        _CACHE["nc"] = _build_nc()
    return _CACHE["nc"]


def _prep_inputs(x, lin_W, lin_b, W1, b1, W2):
    """Host prep: fold the basis chain, split diagonal, pack fp8 tiles."""
    x = np.asarray(x, dtype=np.float32)
    lin_W = np.asarray(lin_W, dtype=np.float32)
    a = np.sum(np.asarray(W1, np.float32) * np.asarray(W2, np.float32),
               axis=1)
    c = np.sum(np.asarray(b1, np.float32) * np.asarray(W2, np.float32),
               axis=1)
    W_eff = np.ascontiguousarray(lin_W.T)
    idx = np.arange(D)
    dvec = W_eff[idx, idx] + a          # full diagonal, applied on host
    W_nd = W_eff.copy()
    W_nd[idx, idx] = 0.0
    bias = np.asarray(lin_b, np.float32) + c

    # Device W: fp8(32 * W_offdiag), packed [p, q, i, f]
    w8 = (W_nd * WSCALE).astype(FP8_NP).reshape(NQ, 2, P, D)
    w8 = np.ascontiguousarray(w8.transpose(2, 0, 1, 3))

    # Device x: fp8(x), packed per core [bt, p, q, i, b]
    x8 = x.astype(FP8_NP).reshape(NCORES, NB, P, NQ, 2, P)
    x8t = np.ascontiguousarray(x8.transpose(0, 1, 5, 3, 4, 2))
    return x8t, w8, x, dvec, bias


def _postprocess(results, x, dvec, bias):
    """Host post: descale the fp8 partial sums, add diagonal + bias."""
    out_dev = np.concatenate([r["out"] for r in results], axis=0)
    out = out_dev.astype(np.float32)
    out *= np.float32(1.0 / WSCALE)
    out += x * dvec
    out += bias
    return np.ascontiguousarray(out.astype(np.float32))


def kernel(x, lin_W, lin_b, W1, b1, W2):
    x8t, w8, x32, dvec, bias = _prep_inputs(x, lin_W, lin_b, W1, b1, W2)
    in_maps = [{"x8t": x8t[i], "w8": w8} for i in range(NCORES)]
    nc = _get_nc()
    res = run_bass_kernel_spmd(nc, in_maps, core_ids=list(range(NCORES)))
    return _postprocess(res.results, x32, dvec, bias)
